# revision 45
# baseline (speedup 1.0000x reference)
"""Trainium2 Bass kernel for nn_BinarySquareClassifier (3-layer LIF SNN).

Strategy (pure data parallel over batch, 8 cores, B=2048 -> 256/core):
- One stacked bf16 matmul per 8-step time-chunk computes h1/h2/h3 for all
  three layers at once with pipeline skew (layer l's inputs come one chunk
  later than layer l-1's outputs), so the three layers' serial LIF scans
  run on time-shifted frames stacked into one [98, 256] membrane state M.
- "m-space" reparameterization: within a chunk, column position k carries
  scale beta^-k (x columns pre-scaled on host, spike columns written with
  the scale folded into the is_gt output, cross-layer weight blocks carry
  an extra beta).  The per-step membrane decay then becomes a plain ADD
  (m += h), the reset a plain SUBTRACT (m -= s~), both bf16 tensor_tensor
  ops that hit the DVE 2x perf mode; the spike op is a fused
  (m > th_k) * c_{k+1} tensor_scalar that hits the 4x mode.  One
  scalar_tensor_tensor per chunk applies the beta^8 basis rescale.
- Biases folded into per-k thresholds th_k = (1 - b/(1-beta)) * beta^-k;
  warmup freezing of layers 2/3 via +BIG threshold phases and pre-decayed
  initial state.
- Scan schedule (timeline-model driven): batch columns split into a
  D-stream [0,HA) spiked on DVE and a P-stream [HA,B) spiked on Pool.
  Each step's DVE work is [add_P, add_D, spikeD(k-1), sub_P, sub_D],
  software-pipelined so every same-engine data dependency is >=2
  instructions back -- adjacent dependent DVE ops pay a ~95ns semaphore
  round (write-drain + sem propagation), one op in between hides it
  completely.  The D-spike of step k-1 is deferred into period k as the
  pipeline filler; both k=7 spike halves run on DVE inside the next g0
  group so the Pool handoff never lands on the rescale boundary's
  critical path.  Steady state is ~602ns/step with DVE ~98% busy.
- ACT bulk-copies each chunk's PSUM H-block to SBUF bf16 off the
  critical path; PE does the bulk matmul in bf16.  Chunk c's matmul
  blk3 consumes cur col-7 spikes produced by the g0 group of the same
  iteration, so blk3/copy1 are emitted after it (dependency direction
  must match program order for the tile scheduler).
- Layer-3 spike rows are stashed per chunk via SBUF->SBUF DMA and reduced
  at the end with bf16 adds + a beta-weighted ones-vector matmul that
  undoes the m-space spike scaling.
"""

import numpy as np
from contextlib import ExitStack

B_FULL, C_IN, T_FULL = 2048, 30, 1024
N_CORES = 8
B = B_FULL // N_CORES           # 256 batch per core
HA = 130                        # batch split: [0,HA) DVE-spiked, [HA,B) Pool
TC = 8                          # timesteps per chunk
N_CHUNKS = T_FULL // TC         # 128
# m-space basis rescale period: MUST equal TC — the layer pipeline skew is
# TC steps, so spike-column scales are only chunk-invariant (and consumable
# by the fixed lhsT beta-fold) when positions repeat with period TC
P = 8
BETA = 0.9
BIG = 3.0e38

_cache = {}


def _split_multi_waits(nc):
    """This container's walrus accepts only ONE sync-wait per instruction;
    hoist extra waits onto same-engine nops inserted just before.  For most
    engines use an ENGINE_NOP (waits park in the engine wait-queue, leaving
    the sequencer free to keep decoding); Pool keeps a sequencer-only NoOp
    because its engine path pays the 95ns Q7 launch per instruction."""
    import concourse.mybir as mybir
    eng_by_type = {
        mybir.EngineType.DVE: nc.vector,
        mybir.EngineType.Activation: nc.scalar,
        mybir.EngineType.PE: nc.tensor,
        mybir.EngineType.SP: nc.sync,
        mybir.EngineType.Pool: nc.gpsimd,
    }
    # Pass 1: drop redundant waits.  Engines execute in order, so once an
    # instruction on engine E has waited for (sem >= v), every later
    # instruction on E sees that condition already satisfied — a repeat wait
    # on the same sem with <= value is a no-op on hardware and pure overhead
    # in the timeline.  Any write-mode sem update (barrier resets)
    # invalidates the assumption for that sem globally.
    ge_mode = "sem-ge-imm"
    seen = {}  # (engine, sem_id) -> max value already waited
    for f in nc.m.functions:
        for blk in f.blocks:
            for inst in blk.instructions:
                si = inst.sync_info
                if si is None:
                    continue
                if si.on_update:
                    for u in si.on_update:
                        if str(u.update_mode) not in ("sem-add-imm", "sem-inc"):
                            for key in [k for k in seen if k[1] == u.id]:
                                del seen[key]
                if not si.on_wait:
                    continue
                kept = []
                for w in si.on_wait:
                    key = (inst.engine, w.id)
                    if (str(w.wait_mode) == ge_mode and w.wait_value is not None
                            and seen.get(key, None) is not None
                            and seen[key] >= w.wait_value):
                        continue  # redundant
                    kept.append(w)
                    if str(w.wait_mode) == ge_mode and w.wait_value is not None:
                        if seen.get(key, -1) < w.wait_value:
                            seen[key] = w.wait_value
                if len(kept) != len(si.on_wait):
                    inst.sync_info = mybir.SyncInfo(
                        on_wait=kept, on_update=list(si.on_update or []))

    counter = 0
    for f in nc.m.functions:
        for blk in f.blocks:
            out = []
            changed = False
            for inst in blk.instructions:
                si = inst.sync_info
                if si is not None and si.on_wait is not None and len(si.on_wait) > 1:
                    waits = list(si.on_wait)
                    for w in waits[:-1]:
                        counter += 1
                        if inst.engine == mybir.EngineType.DVE:
                            nop = eng_by_type[inst.engine]._isa(
                                nc.isa.Opcode.NEURON_ISA_TPB_OPCODE_ENGINE_NOP,
                                {})
                        else:
                            nop = mybir.InstNoOp(
                                name=f"waitsplit-{counter}", ins=[], outs=[])
                        nop.engine = inst.engine
                        nop.sync_info = mybir.SyncInfo(on_wait=[w], on_update=[])
                        out.append(nop)
                    inst.sync_info = mybir.SyncInfo(
                        on_wait=[waits[-1]], on_update=list(si.on_update or []))
                    changed = True
                out.append(inst)
            if changed:
                try:
                    blk.instructions[:] = out
                except TypeError:
                    blk.instructions = out


def _build_program():
    import concourse.bass as bass
    import concourse.mybir as mybir
    import concourse.tile as tile

    nc = bass.Bass("TRN2", target_bir_lowering=False, debug=False,
                   num_devices=N_CORES)
    f32 = mybir.dt.float32
    bf16 = mybir.dt.bfloat16
    AOT = mybir.AluOpType
    BP = float(np.float64(np.float32(BETA)) ** P)
    CK = [float(np.float64(np.float32(BETA)) ** (-g)) for g in range(P + 1)]

    xt_in = nc.dram_tensor("xt", [C_IN, T_FULL, B], bf16, kind="ExternalInput").ap()
    lhsT_in = nc.dram_tensor("lhsT", [128, 98], bf16, kind="ExternalInput").ap()
    th_in = nc.dram_tensor("th", [98, 3 * P + 1], f32, kind="ExternalInput").ap()
    wred_in = nc.dram_tensor("wred", [128, 1], bf16, kind="ExternalInput").ap()
    acc_out = nc.dram_tensor("acc", [1, 512], f32, kind="ExternalOutput").ap()

    with ExitStack() as ctx:
        tc = ctx.enter_context(tile.TileContext(nc))
        pool = ctx.enter_context(tc.tile_pool(name="sb", bufs=1))
        psum_pool = ctx.enter_context(tc.tile_pool(name="ps", bufs=1, space="PSUM"))

        t_lhsT = pool.tile([128, 98], bf16, tag="lhsT", name="lhsT")
        t_th = pool.tile([98, 3 * P + 1], f32, tag="th", name="th")
        # ping-pong membrane state: two buffers, alternating per tau, so the
        # Pool-engine spike read of M[p] never blocks the DVE write of M[1-p]
        t_M = [pool.tile([98, B], bf16, tag=f"M{i}", name=f"M{i}")
               for i in range(2)]
        t_rhs = [pool.tile([128, TC * B], bf16, tag=f"rhs{i}", name=f"rhs{i}")
                 for i in range(2)]
        t_hs = [pool.tile([98, TC * B], bf16, tag=f"Hs{i}", name=f"Hs{i}")
                for i in range(2)]
        t_stash = pool.tile([128, 4096], bf16, tag="stash", name="stash")
        t_wred = pool.tile([128, 1], bf16, tag="wred", name="wred")
        t_part = pool.tile([128, 512], bf16, tag="part", name="part")
        t_accf = pool.tile([1, 512], f32, tag="accf", name="accf")
        t_ps = [psum_pool.tile([98, TC * B], f32, tag=f"H{i}", name=f"H{i}")
                for i in range(2)]
        t_psr = t_ps[1][0:1, 0:512]  # reuse a PSUM bank for the final reduce

        # Startup DMA order matters: the HWDGE serializes descriptor
        # generation (~625ns per DMA), so the chunk-0 critical path
        # (lhsT + x blk0 -> matmul blk0; m0 -> g0 subs) goes first.
        # Only rhs0's spike rows need zeroing (rhs1's are fully written by
        # chunk 0's scan before anything reads them; rows 0:98 also cover
        # the k=0 reset read of the last column). Split per matmul block
        # so the first sub-matmul isn't gated on the whole memset.
        for blk in range(4):
            nc.gpsimd.memset(t_rhs[0][0:98, blk * 512:(blk + 1) * 512], 0.0)

        def xdma0(blk):
            nc.sync.dma_start(
                out=t_rhs[0][98:128, blk * 512:(blk + 1) * 512],
                in_=xt_in[:, blk * 2:(blk + 1) * 2, :].rearrange(
                    "c t b -> c (t b)"),
            )

        # spread across engine DMA queues so descriptor generation for the
        # chunk-0 critical path (x blk0 + lhsT -> matmul; m0 -> g0 subs)
        # runs in parallel rather than serializing on one HWDGE path
        xdma0(0)
        nc.scalar.dma_start(out=t_lhsT[:], in_=lhsT_in[:])
        xdma0(1)
        nc.sync.dma_start(out=t_th[:], in_=th_in[:])
        xdma0(2)
        xdma0(3)
        nc.sync.dma_start(out=t_wred[:], in_=wred_in[:])

        def phase(c):
            return 0 if c == 0 else (1 if c == 1 else 2)

        for c in range(N_CHUNKS + 2):
            cur = t_rhs[c % 2]
            nxt = t_rhs[(c + 1) % 2]
            ps = t_ps[c % 2]
            hs = t_hs[c % 2]
            ph = phase(c)

            # prefetch x for chunk c+1 (overlaps this chunk's scan)
            if c + 1 < N_CHUNKS:
                nc.sync.dma_start(
                    out=nxt[98:128, :],
                    in_=xt_in[:, (c + 1) * TC:(c + 2) * TC, :].rearrange(
                        "c t b -> c (t b)"),
                )

            # stacked bf16 matmul for this chunk: 4 sub-matmuls of 512
            # columns.  Emission order matters for dependency direction:
            # blk3 consumes cur col 7 spikes, which the g0 group below
            # produces, so blk3/copy1 are emitted after it; the g0 group's
            # stt reads hs cols 0:B from copy0, emitted before it.
            for blk in range(3):
                sl = slice(blk * 512, (blk + 1) * 512)
                nc.tensor.matmul(ps[:, sl], t_lhsT[:], cur[0:128, sl])
                if blk == 0:
                    # eager copy: unblocks the g0 stt (cols 0:B)
                    # without waiting for matmul blk1
                    nc.scalar.copy(hs[:, 0:512], ps[:, 0:512])
                elif blk == 1:
                    nc.scalar.copy(hs[:, 512:1024], ps[:, 512:1024])
                else:
                    # cols 1024:1536 need only blk2 - copy before the g0
                    # group so periods 4-5 never wait on the post-g0 blk3
                    nc.scalar.copy(hs[:, 1024:1536], ps[:, 1024:1536])

            # Serial LIF scan in m-space.  Two independent batch-column
            # streams: D = cols [0,HA) spiked on DVE, Pstream = cols [HA,B)
            # spiked on Pool.  The DVE schedule is software-pipelined so
            # every same-engine data dependency is >=2 instructions back
            # (adjacent-dependent DVE ops pay a ~95ns semaphore round; with
            # one op in between the sem is satisfied for free).  The D-spike
            # of step k-1 is emitted inside period k as the pipeline filler;
            # both halves of the k=7 spike run on DVE inside the next g0
            # group (the Pool->DVE handoff would stall across the rescale).
            #
            # g0 group (k=0, basis rescale): reset in the old basis (stored
            # spike scale c_P cancels beta^P exactly), then the rescale
            # fused with the h-add via scalar_tensor_tensor.
            th_p = t_th[:, phase(c - 1) * P + (P - 1):phase(c - 1) * P + P]
            M7 = t_M[0]                 # M of k=7 (odd parity -> buffer 0)
            Mp, Mc = t_M[0], t_M[1]
            r7 = (TC - 1) * B
            if c > 0:
                # previous chunk's k=7 spikes -> cur col 7 (th/ck of g=7)
                nc.vector.tensor_scalar(
                    cur[0:98, r7 + HA:r7 + B], M7[:, HA:B], th_p, CK[P],
                    AOT.is_gt, AOT.mult)
                nc.vector.tensor_scalar(
                    cur[0:98, r7:r7 + HA], M7[:, 0:HA], th_p, CK[P],
                    AOT.is_gt, AOT.mult)
            if c > 0:
                nc.vector.tensor_tensor(
                    Mc[:, HA:B], Mp[:, HA:B], cur[0:98, r7 + HA:r7 + B],
                    AOT.subtract)
                nc.vector.tensor_tensor(
                    Mc[:, 0:HA], Mp[:, 0:HA], cur[0:98, r7:r7 + HA],
                    AOT.subtract)
            if c > 0:
                nc.vector.scalar_tensor_tensor(
                    Mc[:, HA:B], Mc[:, HA:B], BP, hs[:, HA:B],
                    AOT.mult, AOT.add)
                nc.vector.scalar_tensor_tensor(
                    Mc[:, 0:HA], Mc[:, 0:HA], BP, hs[:, 0:HA],
                    AOT.mult, AOT.add)
            else:
                # chunk-0 init: m0 is columnwise-constant, so M_0 = m0*b^P + h
                # is a per-partition scalar add (m0*b^P rides as th col 3P;
                # no [98,B] m0 DMA on the cold-start critical path)
                m0c = t_th[:, 3 * P:3 * P + 1]
                nc.vector.tensor_scalar(
                    Mc[:, HA:B], hs[:, HA:B], m0c, 1.0, AOT.add, AOT.mult)
                nc.vector.tensor_scalar(
                    Mc[:, 0:HA], hs[:, 0:HA], m0c, 1.0, AOT.add, AOT.mult)
            th_0 = t_th[:, ph * P:ph * P + 1]
            nc.gpsimd.tensor_scalar(
                nxt[0:98, HA:B], Mc[:, HA:B], th_0, CK[1], AOT.is_gt, AOT.mult)

            # stash layer-3 spike rows of chunk c-1 (complete as of this
            # iteration's g0 group), i.e. output frame f = c-3
            f = c - 3
            if 0 <= f < N_CHUNKS:
                p0 = (f % 16) * TC
                cb = f // 16
                for j in range(2):
                    nc.sync.dma_start(
                        out=t_stash[p0:p0 + TC,
                                    cb * 512 + j * B:cb * 512 + (j + 1) * B],
                        in_=cur[96 + j:97 + j, :],
                    )

            # blk3 + copy1, now that cur col 7 spikes exist in program order
            nc.tensor.matmul(ps[:, 1536:2048], t_lhsT[:], cur[0:128, 1536:2048])
            nc.scalar.copy(hs[:, 1536:2048], ps[:, 1536:2048])

            # periods k=1..7: [add_P, add_D, spD(k-1), sub_P, sub_D] (+ Pool
            # spike for k<7).  M ping-pong: M_k lives in t_M[1-k%2].
            for k in range(1, TC):
                g = k                  # TC == P, so g == k
                th_k = t_th[:, ph * P + g:ph * P + g + 1]
                th_km = t_th[:, ph * P + g - 1:ph * P + g]
                Mp = t_M[k % 2]        # M_{k-1}
                Mc = t_M[1 - k % 2]    # M_k
                col0 = k * B
                rc = (k - 1) * B
                nc.vector.tensor_tensor(
                    Mc[:, HA:B], Mp[:, HA:B], hs[:, col0 + HA:col0 + B],
                    AOT.add)
                nc.vector.tensor_tensor(
                    Mc[:, 0:HA], Mp[:, 0:HA], hs[:, col0:col0 + HA], AOT.add)
                # D-spike of step k-1 (reads M_{k-1} = Mp buffer)
                nc.vector.tensor_scalar(
                    nxt[0:98, rc:rc + HA], Mp[:, 0:HA], th_km, CK[g],
                    AOT.is_gt, AOT.mult)
                nc.vector.tensor_tensor(
                    Mc[:, HA:B], Mc[:, HA:B], nxt[0:98, rc + HA:rc + B],
                    AOT.subtract)
                nc.vector.tensor_tensor(
                    Mc[:, 0:HA], Mc[:, 0:HA], nxt[0:98, rc:rc + HA],
                    AOT.subtract)
                if k < TC - 1:
                    nc.gpsimd.tensor_scalar(
                        nxt[0:98, col0 + HA:col0 + B], Mc[:, HA:B], th_k,
                        CK[g + 1], AOT.is_gt, AOT.mult)


        # flush the last drain chunk's k=7 spikes and stash its frame
        fin = t_rhs[N_CHUNKS % 2]      # nxt of chunk N_CHUNKS+1
        th_f = t_th[:, 2 * P + (P - 1):2 * P + P]
        r7 = (TC - 1) * B
        nc.vector.tensor_scalar(
            fin[0:98, r7 + HA:r7 + B], t_M[0][:, HA:B], th_f, CK[P],
            AOT.is_gt, AOT.mult)
        nc.vector.tensor_scalar(
            fin[0:98, r7:r7 + HA], t_M[0][:, 0:HA], th_f, CK[P],
            AOT.is_gt, AOT.mult)
        f = N_CHUNKS - 1
        p0 = (f % 16) * TC
        cb = f // 16
        for j in range(2):
            nc.sync.dma_start(
                out=t_stash[p0:p0 + TC,
                            cb * 512 + j * B:cb * 512 + (j + 1) * B],
                in_=fin[96 + j:97 + j, :],
            )

        # reduce stash: sum the 8 column blocks (bf16), then the weighted
        # ones-vector matmul over partitions undoes the m-space spike scale
        nc.vector.tensor_tensor(
            t_part[:], t_stash[:, 0:512], t_stash[:, 512:1024], AOT.add)
        for cb in range(2, 8):
            nc.vector.tensor_tensor(
                t_part[:], t_part[:], t_stash[:, cb * 512:(cb + 1) * 512], AOT.add)
        nc.tensor.matmul(t_psr[:], t_wred[:], t_part[:])
        nc.scalar.copy(t_accf[:], t_psr[:])
        nc.sync.dma_start(out=acc_out[:], in_=t_accf[:])

    _split_multi_waits(nc)
    return nc


def _host_consts(W1, b1, W2, b2, W3, b3):
    import ml_dtypes
    bf16 = ml_dtypes.bfloat16
    beta64 = np.float64(np.float32(BETA))

    # lhsT: x-block carries W1 (x columns pre-scaled by c_k on host); spike
    # columns carry scale c_{k+1} = c_k / beta, so cross-layer blocks get
    # an extra beta.
    lhsT = np.zeros((128, 98), np.float64)
    lhsT[98:128, 0:64] = W1.T
    lhsT[0:64, 64:96] = beta64 * W2.T
    lhsT[64:96, 96:98] = beta64 * W3.T
    lhsT = lhsT.astype(bf16)

    c1 = (b1.astype(np.float64) / (1.0 - beta64))
    c2 = (b2.astype(np.float64) / (1.0 - beta64))
    c3 = (b3.astype(np.float64) / (1.0 - beta64))
    th_main = np.concatenate([1.0 - c1, 1.0 - c2, 1.0 - c3])  # [98]
    th_w0 = th_main.copy(); th_w0[64:98] = BIG
    th_w1 = th_main.copy(); th_w1[96:98] = BIG
    # per-position thresholds th * beta^-g for g=0..P-1, per warmup phase
    th = np.zeros((98, 3 * P), np.float64)
    for p, base in enumerate([th_w0, th_w1, th_main]):
        for g in range(P):
            th[:, p * P + g] = np.minimum(base * beta64 ** (-g), BIG)
    th = th.astype(np.float32)

    # m0 = Mhat(-1) * beta^-(P-1)  (see g=0 stt: m = beta^P*m + h)
    m0 = np.zeros(98, np.float64)
    m0[0:64] = -c1
    m0[64:96] = -c2 * beta64 ** (-TC)
    m0[96:98] = -c3 * beta64 ** (-2 * TC)
    m0 *= beta64 ** (-(P - 1))
    BP = beta64 ** P
    th = np.concatenate([th, (m0 * BP).astype(np.float32)[:, None]], axis=1)

    # weighted ones for the final stash reduce: stash partition p holds
    # global position g = p mod 32 spikes with value beta^-(g+1)
    wred = (beta64 ** (np.arange(128) % P + 1)).astype(bf16).reshape(128, 1)
    return lhsT, th, wred


def kernel(x, W1, b1, W2, b2, W3, b3):
    import ml_dtypes
    from concourse.bass_utils import run_bass_kernel_spmd
    bf16 = ml_dtypes.bfloat16

    x = np.asarray(x, np.float32)
    W1 = np.asarray(W1, np.float32); b1 = np.asarray(b1, np.float32)
    W2 = np.asarray(W2, np.float32); b2 = np.asarray(b2, np.float32)
    W3 = np.asarray(W3, np.float32); b3 = np.asarray(b3, np.float32)

    if "nc" not in _cache:
        _cache["nc"] = _build_program()
    nc = _cache["nc"]

    lhsT, th, wred = _host_consts(W1, b1, W2, b2, W3, b3)
    # x scaled by c_{t mod P} = beta^-(t mod P) per the m-space convention
    tscale = (np.float64(np.float32(BETA)) **
              (-(np.arange(T_FULL) % P))).astype(np.float32)
    in_maps = []
    for core in range(N_CORES):
        xs = x[core * B:(core + 1) * B]                # [256, 30, 1024]
        xt = np.transpose(xs, (1, 2, 0))               # [30, 1024, 256]
        xt = np.ascontiguousarray(xt * tscale[None, :, None]).astype(bf16)
        in_maps.append({"xt": xt, "lhsT": lhsT, "th": th, "wred": wred})

    res = run_bass_kernel_spmd(nc, in_maps, list(range(N_CORES)))
    out = np.empty((B_FULL, 2), np.float32)
    for core in range(N_CORES):
        a = res.results[core]["acc"].reshape(2, 256)   # [j, b]
        out[core * B:(core + 1) * B] = a.T
    return out

